# revision 44
# baseline (speedup 1.0000x reference)
"""Trainium2 Bass kernel: 5x5 reflect-padded box-filter mean (LocalMean).

Full input:  image (32, 3, 512, 512) f32
Full output: same shape; out[r,c] = mean of the 5x5 window of the
reflect-padded image.

Strategy (pure data parallel over 8 NeuronCores, 4 images per core):
- Host pre-pads H and W by 2 with reflect, lays the tensor out as
  [HP, PB, C*WP] in bf16: rows outermost, so ONE dma_start per row-block
  loads all 4 images x 3 channels with 12.4 KB contiguous per-partition
  descriptors (input HBM traffic also halves vs f32).
- On-chip the filter is separable:
  * vertical 5-tap sum via TensorE banded matmuls in bf16 (1 cycle/row
    vs 4 for fp32), weight band pre-scaled by 1/25; row blocks of 124
    output rows so the 128 input rows fit one SBUF tile,
  * ScalarE copies PSUM f32 -> SBUF bf16 (the only PSUM drain),
  * horizontal 5-tap sum via ONE DVE tensor_tensor_scan per (block,
    image) running across all 3 channels back-to-back: the recurrence
    H[t] = H[t-1] + V[t] - V[t-5] telescopes exactly (fp32 internal
    state), so window sums spanning a channel boundary are garbage but
    self-cancel and are simply never stored.
- Output stays bf16 in [H, PB, C*W] layout (ONE dma_start per row-block
  covering all 4 images) and is upcast/reordered on host. Total HBM
  traffic/core ~12.7 MB vs ~25.4 MB for f32, in 10 DMAs per pass.
"""

import numpy as np

N_CORES = 8
B, C, H, W = 32, 3, 512, 512
PB = B // N_CORES          # images per core
PAD = 2
HP, WP = H + 2 * PAD, W + 2 * PAD   # 516
FW = C * WP                # 1548: per-image in-tile free width
FO = C * W                 # 1536: per-image out free width

# Output-row blocks of 124 (last 16): input rows [r0, r0+h+4) per block
# sit in one 128-partition tile, so the vertical matmul needs no
# cross-tile tail accumulation.
BLOCKS = [(0, 124), (124, 124), (248, 124), (372, 124), (496, 16)]

_CACHE = {}
# Experiment switches (default = the shipped configuration). Ablation
# flags (no_*) produce WRONG results and exist only for HW bottleneck
# timing via bench3.py.
# Shipped defaults: output DMAs ride the SWDGE (gpsimd) path, which
# sustains ~135-150 GB/s for SBUF->HBM stores vs ~93 GB/s on HWDGE
# (HW-measured), split 2-ways per row-block so each half flies as soon
# as its two images' scans finish.
_CFG = {"odma_gpsimd": True, "odma_split": 2}


def _band_weights():
    # W[k, m] = 1/25 for 0 <= k-m <= 4: vertical 5-tap window starting at
    # output row m reads input rows m..m+4 of the padded block.
    def band(K, M):
        k = np.arange(K)[:, None]
        m = np.arange(M)[None, :]
        return (((k - m) >= 0) & ((k - m) <= 4)).astype(np.float32) / 25.0
    return band(128, 124), band(20, 16)


def _build(reps=1, loop_n=None):
    # loop_n: wrap ONE rep in a hardware For_i loop executing loop_n
    # times (bench-only: tiny NEFF, on-device repetition for high-SNR
    # timing). reps: python-unrolled repetitions (the graded/test path).
    import concourse.bacc as bacc
    import concourse.tile as tile
    from concourse import mybir

    f32 = mybir.dt.float32
    bf16 = mybir.dt.bfloat16
    nc = bacc.Bacc("TRN2", target_bir_lowering=False, debug=False,
                   num_devices=N_CORES)
    x = nc.dram_tensor("x", [HP, PB, FW], bf16, kind="ExternalInput").ap()
    wd = nc.dram_tensor("wd", [128, 124], bf16, kind="ExternalInput").ap()
    wl = nc.dram_tensor("wl", [20, 16], bf16, kind="ExternalInput").ap()
    # Output keeps the on-chip padded layout (garbage cols included!):
    # a fully-contiguous store needs only 128 fat descriptors per DMA,
    # vs 1488 gap-fragmented 1KB ones which run at ~85 GB/s on HW.
    # Host strips the pad columns afterwards.
    y = nc.dram_tensor("y", [H, PB * FW], bf16, kind="ExternalOutput").ap()
    if _CFG.get("odma_internal"):
        y = nc.dram_tensor("yint", [H, PB * FW], bf16, kind="Internal").ap()

    LOOKAHEAD = _CFG.get("lookahead", 2)  # row-blocks prefetched

    with tile.TileContext(nc) as tc:
        with (
            tc.tile_pool(name="wp", bufs=1) as wp,
            tc.tile_pool(name="xp", bufs=LOOKAHEAD + 2) as xp,
            tc.tile_pool(name="vp", bufs=2, space="PSUM") as vp,
            tc.tile_pool(name="vp2", bufs=_CFG.get("vp2b", 4),
                         space="PSUM") as vp2,
            tc.tile_pool(name="vsp", bufs=_CFG.get("vspb", 4)) as vsp,
            tc.tile_pool(name="fvsp", bufs=_CFG.get("fvspb", 3)) as fvsp,
            tc.tile_pool(name="op", bufs=_CFG.get("opb", 3)) as op,
        ):
            d_t = wp.tile([128, 124], bf16)
            nc.sync.dma_start(d_t[:], wd[:, :])
            l_t = wp.tile([20, 16], bf16)
            nc.sync.dma_start(l_t[:], wl[:, :])

            nb = len(BLOCKS)
            steps = list(range((reps if loop_n is None else 1) * nb))
            loaded = {}  # step index -> X tile (one row-block, 4 images)

            def load(s):
                r0, h = BLOCKS[s % nb]
                kh = h + 4
                t = xp.tile([128, PB * FW], bf16)
                if _CFG.get("no_idma0"):
                    # Zero-DMA ablation; tiny memset keeps the tile "written"
                    # so the Tile release pass doesn't assert.
                    nc.vector.memset(t[0:1, 0:4], 0.25)
                elif _CFG.get("idma_half"):
                    # Ablation: half the input HBM traffic; upper tile rows
                    # keep the previous pool user's (sane) data.
                    nc.sync.dma_start(t[0:kh // 2, :],
                                      x[r0:r0 + kh // 2, :, :])
                else:
                    isplit = _CFG.get("idma_split", 1)
                    istep = PB // isplit
                    for n0 in range(0, PB, istep):
                        nc.sync.dma_start(
                            t[0:kh, n0 * FW:(n0 + istep) * FW],
                            x[r0:r0 + kh, n0:n0 + istep, :])
                loaded[s] = t

            def emit_body():
              for s in steps[:min(LOOKAHEAD, len(steps))]:
                load(s)

              for s in steps:
                if s + LOOKAHEAD < len(steps):
                    load(s + LOOKAHEAD)
                xt = loaded.pop(s)
                r0, h = BLOCKS[s % nb]
                kh = h + 4
                w_t = d_t if h == 124 else l_t

                o = op.tile([128, PB * FW], bf16)
                if _CFG.get("no_dve0"):
                    nc.vector.memset(o[0:1, 0:4], 0.25)
                osplit = _CFG.get("odma_split", 1)
                ostep = PB // osplit

                def odma(n0, step):
                    if _CFG.get("no_odma0"):
                        return
                    if _CFG.get("no_odma"):
                        nc.sync.dma_start(y[0:1, n0:n0 + 1],
                                          o[0:1, n0:n0 + 1])
                        return
                    nring = _CFG.get("odma_rings", 1)
                    if _CFG.get("odma_mix"):
                        # Alternate writes between the HWDGE (SP) and the
                        # faster SWDGE (gpsimd) paths so both write streams
                        # progress concurrently.
                        rings = [nc.gpsimd, nc.sync]
                        odma_eng = rings[(s * osplit + n0 // step)
                                         % len(rings)]
                    elif nring > 1:
                        rings = [nc.sync, nc.scalar, nc.gpsimd][:nring]
                        odma_eng = rings[(s * osplit + n0 // step)
                                         % len(rings)]
                    else:
                        odma_eng = (nc.gpsimd if _CFG.get("odma_gpsimd")
                                    else nc.scalar if _CFG.get("odma_act")
                                    else nc.sync)
                    odma_eng.dma_start(
                        y[r0:r0 + h, n0 * FW:(n0 + step) * FW],
                        o[0:h, n0 * FW:(n0 + step) * FW])

                fvs = (fvsp.tile([128, PB * FW], bf16, name="fvs")
                       if _CFG.get("fuse_scan") else None)
                for n in range(PB):
                    x0 = n * FW
                    # V[m, t] = sum_{d=0..4} X[m+d, t] / 25 via banded
                    # matmul; N split at PSUM bank boundaries (512 f32).
                    if _CFG.get("psum_split"):
                        # Two half-width 2-bank PSUM tiles (pool depth 4):
                        # the PE can run further ahead of the ACT drain,
                        # avoiding p-state cold-starts from back-pressure.
                        HWID = FW // 2  # 774
                        vparts = []
                        for half in range(2):
                            vh = vp2.tile([128, HWID], f32, name=f"vh{half}")
                            b0 = half * HWID
                            if not _CFG.get("no_pe"):
                                for c0 in range(b0, b0 + HWID, 512):
                                    c1 = min(c0 + 512, b0 + HWID)
                                    nc.tensor.matmul(
                                        vh[0:h, c0 - b0:c1 - b0],
                                        w_t[0:kh, 0:h],
                                        xt[0:kh, x0 + c0:x0 + c1],
                                        start=True, stop=True)
                            vparts.append(vh)
                        dsts = [(0, HWID), (HWID, FW)]
                    else:
                        v = vp.tile([128, FW], f32)
                        if _CFG.get("no_pe0"):
                            if not _CFG.get("no_act0"):
                                nc.vector.memset(v[0:1, 0:4], 0.25)
                        elif not _CFG.get("no_pe"):
                            for c0 in range(0, FW, 512):
                                c1 = min(c0 + 512, FW)
                                nc.tensor.matmul(v[0:h, c0:c1],
                                                 w_t[0:kh, 0:h],
                                                 xt[0:kh, x0 + c0:x0 + c1],
                                                 start=True, stop=True)
                        else:
                            nc.tensor.matmul(v[0:h, 0:1], w_t[0:kh, 0:h],
                                             xt[0:kh, x0:x0 + 1],
                                             start=True, stop=True)

                    # Single PSUM drain, f32 -> bf16 (scan operands must
                    # not both be in PSUM; DVE reads SBUF cheaper anyway).
                    if fvs is not None:
                        if not _CFG.get("no_act0"):
                            if _CFG.get("psum_split"):
                                for vh, (d0, d1) in zip(vparts, dsts):
                                    nc.scalar.copy(
                                        fvs[0:h, x0 + d0:x0 + d1],
                                        vh[0:h, :])
                            else:
                                nc.scalar.copy(fvs[0:h, x0:x0 + FW],
                                               v[0:h, :])
                        elif not _CFG.get("no_dve0"):
                            nc.vector.memset(fvs[0:1, x0:x0 + 4], 0.25)
                        continue
                    vs = vsp.tile([128, FW], bf16)
                    if _CFG.get("no_act0"):
                        if not _CFG.get("no_dve0"):
                            nc.vector.memset(vs[0:1, 0:4], 0.25)
                    elif _CFG.get("psum_split"):
                        for vh, (d0, d1) in zip(vparts, dsts):
                            nc.scalar.copy(vs[0:h, d0:d1], vh[0:h, :])
                    elif not _CFG.get("no_act"):
                        nc.scalar.copy(vs[0:h, :], v[0:h, :])
                    else:
                        nc.scalar.copy(vs[0:h, 0:1], v[0:h, 0:1])

                    # Horizontal 5-tap sliding window, one scan across all
                    # 3 channels: o[t] = sum(vs[t-4..t]); channel c's valid
                    # outputs are cols 516c+4 .. 516c+515.
                    if not _CFG.get("fuse_scan"):
                        with nc.allow_low_precision(
                                reason="5-tap window sums; scan state is fp32 "
                                       "internally, tol is 2e-2"):
                            if _CFG.get("no_dve0"):
                                pass
                            elif not _CFG.get("no_dve"):
                                nc.vector.reduce_sum(o[0:h, x0 + 4:x0 + 5],
                                                     vs[0:h, 0:5],
                                                     axis=mybir.AxisListType.X)
                                nc.vector.tensor_tensor_scan(
                                    o[0:h, x0 + 5:x0 + FW], vs[0:h, 5:FW],
                                    vs[0:h, 0:FW - 5], o[0:h, x0 + 4:x0 + 5],
                                    mybir.AluOpType.add,
                                    mybir.AluOpType.subtract)
                            else:
                                nc.vector.reduce_sum(o[0:h, x0 + 4:x0 + 5],
                                                     vs[0:h, 0:5],
                                                     axis=mybir.AxisListType.X)
                                nc.vector.tensor_tensor_scan(
                                    o[0:h, x0 + 5:x0 + 6], vs[0:h, 5:6],
                                    vs[0:h, 0:1], o[0:h, x0 + 4:x0 + 5],
                                    mybir.AluOpType.add,
                                    mybir.AluOpType.subtract)

                    if (n + 1) % ostep == 0 and not _CFG.get("fuse_scan"):
                        odma(n + 1 - ostep, ostep)

                if _CFG.get("fuse_scan"):
                    # ONE scan per block across all 4 images x 3 channels:
                    # garbage at image/channel boundaries telescopes away
                    # within 5 steps and is never stored. Requires the
                    # per-image vs tiles to be one contiguous tile.
                    FB = PB * FW
                    with nc.allow_low_precision(
                            reason="5-tap window sums; scan state is fp32 "
                                   "internally, tol is 2e-2"):
                        if not _CFG.get("no_dve0"):
                            nc.vector.reduce_sum(o[0:h, 4:5],
                                                 fvs[0:h, 0:5],
                                                 axis=mybir.AxisListType.X)
                            nc.vector.tensor_tensor_scan(
                                o[0:h, 5:FB], fvs[0:h, 5:FB],
                                fvs[0:h, 0:FB - 5], o[0:h, 4:5],
                                mybir.AluOpType.add,
                                mybir.AluOpType.subtract)
                    for n0 in range(0, PB, ostep):
                        odma(n0, ostep)

            if loop_n is not None:
                with tc.For_i(0, loop_n, 1):
                    emit_body()
            else:
                emit_body()

    nc.compile()
    return nc


def _get_nc(reps=1, loop_n=None):
    key = ("nc", reps, loop_n)
    if key not in _CACHE:
        _CACHE[key] = _build(reps, loop_n=loop_n)
    return _CACHE[key]


def _shard_inputs(image: np.ndarray):
    import ml_dtypes

    image = np.ascontiguousarray(np.asarray(image, dtype=np.float32))
    padded = np.pad(image, ((0, 0), (0, 0), (PAD, PAD), (PAD, PAD)),
                    mode="reflect")
    # [B, C, HP, WP] -> [HP, B, C, WP] bf16
    ph = padded.transpose(2, 0, 1, 3).astype(ml_dtypes.bfloat16)
    d, dl = _band_weights()
    d = d.astype(ml_dtypes.bfloat16)
    dl = dl.astype(ml_dtypes.bfloat16)
    in_maps = []
    for i in range(N_CORES):
        xi = np.ascontiguousarray(ph[:, i * PB:(i + 1) * PB]) \
            .reshape(HP, PB, FW)
        in_maps.append({"x": xi, "wd": d, "wl": dl})
    return in_maps


def kernel(image: np.ndarray) -> np.ndarray:
    from concourse import bass_utils

    nc = _get_nc()
    in_maps = _shard_inputs(image)
    res = bass_utils.run_bass_kernel_spmd(nc, in_maps,
                                          core_ids=list(range(N_CORES)))
    # per core y: [H, PB*FW] bf16 (padded cols included) -> strip the
    # 4 leading pad cols per channel -> [PB, C, H, W] f32
    outs = []
    for i in range(N_CORES):
        yi = np.asarray(res.results[i]["y"]).astype(np.float32)
        yi = yi.reshape(H, PB, C, WP)[:, :, :, 4:WP]
        outs.append(yi.transpose(1, 2, 0, 3))
    return np.ascontiguousarray(np.concatenate(outs, axis=0))
